# revision 23
# baseline (speedup 1.0000x reference)
"""Single-query attention ("context inner product") on 8 trn2 NeuronCores.

    scores  = enc @ dec[0]          enc: [S=16384, H=2048], dec: [1, H]
    weights = softmax(scores)
    context = weights @ enc         -> [1, H]

Sharding: enc split along seq_len across 8 cores (2048 rows each).

v3 design (from perfetto evidence on the v1 baseline, 67.7 us):
  v1 read f32 enc (16 MB/core) through casting SWDGE DMAs (~210 GB/s)
  and ran its matmuls at HAM-throttled clock: DMA ended ~49 us, PE busy
  59 us -> 67.7 us total.

  v3 stages enc in HBM as fp16 (host-side cast; numerically identical to
  v1's in-DMA cast): 8 MB/core of plain HWDGE loads issued on the idle
  Sync engine's ring -> FIFO, staggered per-block completion every
  ~1.5 us at full HBM rate. Per 128-row block the score work is split so
  no engine exceeds the DMA cadence (walrus rejects fused stt on Pool,
  so gpsimd only gets TensorTensor work):
    A blocks: DVE fused scalar_tensor_tensor (mul+row-sum in one pass)
    B blocks: DVE tensor_mul (fp16 2x) + ACT copy-accum row-sum (2048)
    E blocks: DVE tensor_mul + gpsimd add-halves + ACT row-sum (1024)
  exp on ACT (partially batched) writes w into one w_all[128,16] tile;
  PE runs 4 fp16 matmuls per block (f32 PSUM accum across blocks), with
  8 dummy warm-up matmuls at t=0 so the HAM clock-gate is released
  (1.2 -> 2.4 GHz) before real work arrives. No per-block norm matmul:
  the normalizer partial is just a column-sum of w_all (DVE reduce),
  returned as [128,1] and finished on host. dec is broadcast across
  partitions by a stride-0 HBM DMA on the ACT ring. Block 15 is loaded
  as two half-width DMAs and fused via exp(sc_b + bias=sc_a) so the
  tail dependency chain after the last byte is short.

Host combine: context = (sum_c ctx_c) / (sum_c sum_p wsum_c[p]).
"""

import numpy as np

S, H = 16384, 2048
N_CORES = 8
S_LOCAL = S // N_CORES  # 2048
P = 128                 # SBUF partitions
N_BLOCKS = S_LOCAL // P  # 16 blocks of 128 rows
HB = 512                # f32 elements per PSUM bank
N_BANKS = H // HB       # 4
HH = H // 2             # half-width for the tail block
HQ = H // 4

CLS_A = {0, 5}                              # DVE fused stt
CLS_B = {1, 4, 8, 11, 14}                   # DVE mul + ACT full reduce
CLS_E = {2, 3, 6, 7, 9, 10, 12, 13}         # DVE mul + Pool halve + ACT reduce
LAST = N_BLOCKS - 1                         # two DVE half-stt + biased exp
# exp batching: [blocks] -> one ACT Exp instruction per group. Pairs only:
# wider groups make the PE wait >3.4us and HAM re-throttles the clock.
EXP_GROUPS = [[0], [1], [2, 3], [4, 5], [6, 7], [8, 9], [10, 11], [12, 13], [14]]

_CACHE: dict = {}


def _build():
    import concourse.bacc as bacc
    import concourse.tile as tile
    from concourse import mybir

    f32 = mybir.dt.float32
    f16 = mybir.dt.float16
    nc = bacc.Bacc(
        "TRN2", target_bir_lowering=False, debug=False, num_devices=N_CORES
    )
    enc = nc.dram_tensor("enc", [S_LOCAL, H], f16, kind="ExternalInput").ap()
    dec = nc.dram_tensor("dec", [1, H], f16, kind="ExternalInput").ap()
    out_d = nc.dram_tensor("out", [1, H], f32, kind="ExternalOutput").ap()
    wsum_d = nc.dram_tensor("wsum", [P, 1], f32, kind="ExternalOutput").ap()

    # map block -> exp group; group g's sc tile holds len(group) columns
    grp_of = {}
    for g, blocks in enumerate(EXP_GROUPS):
        for j, b in enumerate(blocks):
            grp_of[b] = (g, j)

    with tile.TileContext(nc) as tc:
        with (
            tc.tile_pool(name="singles", bufs=1) as singles,
            tc.tile_pool(name="enc_pool", bufs=N_BLOCKS - 1) as enc_pool,
            tc.tile_pool(name="half_pool", bufs=2) as half_pool,
            tc.tile_pool(name="prod_pool", bufs=3) as prod_pool,
            tc.tile_pool(name="h1_pool", bufs=3) as h1_pool,
            tc.tile_pool(name="dump_pool", bufs=2) as dump_pool,
            tc.tile_pool(name="dump1k_pool", bufs=2) as dump1k_pool,
            tc.tile_pool(name="small", bufs=4) as small,
            tc.tile_pool(name="psum", bufs=1, space="PSUM") as psum_pool,
        ):
            # dec [1,H] load on the ACT ring, then broadcast across partitions
            # with 4 PE outer products (ones[1,128]^T @ dec-bank). The outer
            # products plus bridge matmuls below also serve as the HAM warm-up
            # that releases the PE clock gate (1.2 -> 2.4 GHz) before real
            # matmul work arrives.
            dec_sb = singles.tile([1, H], f16)
            nc.scalar.dma_start(out=dec_sb[:], in_=dec[:])
            ones_row = singles.tile([1, P], f16)
            nc.vector.memset(ones_row[:], 1.0)
            dec_b = singles.tile([P, H], f16)
            bc = [
                psum_pool.tile([P, HB], f32, tag=f"bc{j}", name=f"bc{j}")
                for j in range(N_BANKS)
            ]
            for b in range(N_BANKS):
                nc.tensor.matmul(
                    bc[b][:], ones_row[:], dec_sb[:, b * HB : (b + 1) * HB],
                    start=True, stop=True,
                )
            for b in range(N_BANKS):
                eng = nc.vector.tensor_copy if b % 2 == 0 else nc.scalar.copy
                eng(dec_b[:, b * HB : (b + 1) * HB], bc[b][:])
            # bridge matmuls: keep the PE busy (junk into bc0) until block 0's
            # weights are ready, so HAM stays un-throttled.
            for _ in range(10):
                nc.tensor.matmul(
                    bc[0][:], ones_row[:], dec_sb[:, :HB], start=True, stop=True
                )

            # enc block loads, all queued on the Sync ring (FIFO -> staggered
            # completion in block order). Block 15 split into two halves.
            enc_t = []
            for i in range(N_BLOCKS - 1):
                t = enc_pool.tile([P, H], f16, tag="enc_t", name=f"enc{i}")
                nc.sync.dma_start(out=t[:], in_=enc[i * P : (i + 1) * P, :])
                enc_t.append(t)
            hA = half_pool.tile([P, HH], f16, tag="hA", name="encL_a")
            hB = half_pool.tile([P, HH], f16, tag="hB", name="encL_b")
            nc.sync.dma_start(out=hA[:], in_=enc[LAST * P :, :HH])
            nc.sync.dma_start(out=hB[:], in_=enc[LAST * P :, HH:])

            w_all = singles.tile([P, N_BLOCKS], f16)
            sc_grp = [
                singles.tile([P, len(blocks)], f32, name=f"scg{g}")
                for g, blocks in enumerate(EXP_GROUPS)
            ]
            ctx_psum = [
                psum_pool.tile([1, HB], f32, tag=f"ctxb{j}", name=f"ctxb{j}")
                for j in range(N_BANKS)
            ]

            def emit_score(i):
                g, j = grp_of[i]
                sc = sc_grp[g][:, j : j + 1]
                e = enc_t[i][:]
                if i in CLS_A:
                    prod = prod_pool.tile([P, H], f16, tag="prod", name="prod")
                    nc.vector.scalar_tensor_tensor(
                        out=prod[:], in0=e, scalar=1.0, in1=dec_b[:],
                        op0=mybir.AluOpType.mult, op1=mybir.AluOpType.mult,
                        accum_out=sc,
                    )
                elif i in CLS_B:
                    prod = prod_pool.tile([P, H], f16, tag="prod", name="prod")
                    nc.vector.tensor_mul(prod[:], e, dec_b[:])
                    dump = dump_pool.tile([P, H], f16, tag="dump", name="dump")
                    nc.scalar.activation(
                        out=dump[:], in_=prod[:],
                        func=mybir.ActivationFunctionType.Copy,
                        accum_out=sc,
                    )
                else:  # CLS_E
                    prod = prod_pool.tile([P, H], f16, tag="prod", name="prod")
                    nc.vector.tensor_mul(prod[:], e, dec_b[:])
                    h1 = h1_pool.tile([P, HH], f16, tag="h1", name="h1")
                    nc.gpsimd.tensor_add(h1[:], prod[:, :HH], prod[:, HH:])
                    dump = dump1k_pool.tile([P, HH], f16, tag="d1k", name="d1k")
                    nc.scalar.activation(
                        out=dump[:], in_=h1[:],
                        func=mybir.ActivationFunctionType.Copy,
                        accum_out=sc,
                    )

            def emit_mms(i):
                first, last = (i == 0), (i == LAST)
                w = w_all[:, i : i + 1]
                for b in range(N_BANKS):
                    if last:
                        src = hA if b < 2 else hB
                        rhs = src[:, (b % 2) * HB : (b % 2 + 1) * HB]
                    else:
                        rhs = enc_t[i][:, b * HB : (b + 1) * HB]
                    nc.tensor.matmul(
                        ctx_psum[b][:], w, rhs, start=first, stop=last
                    )

            for g, blocks in enumerate(EXP_GROUPS):
                for i in blocks:
                    emit_score(i)
                lo, hi = blocks[0], blocks[-1]
                nc.scalar.activation(
                    out=w_all[:, lo : hi + 1], in_=sc_grp[g][:],
                    func=mybir.ActivationFunctionType.Exp,
                )
                for i in blocks:
                    emit_mms(i)

            # tail block 15: two half-width fused score passes, biased exp
            sc_a = small.tile([P, 1], f32, tag="sc_a", name="sc_a")
            sc_b = small.tile([P, 1], f32, tag="sc_b", name="sc_b")
            pd = prod_pool.tile([P, HH], f16, tag="pdead", name="pd")
            nc.vector.scalar_tensor_tensor(
                out=pd[:], in0=hA[:], scalar=1.0, in1=dec_b[:, :HH],
                op0=mybir.AluOpType.mult, op1=mybir.AluOpType.mult,
                accum_out=sc_a[:],
            )
            pd2 = prod_pool.tile([P, HH], f16, tag="pdead", name="pd2")
            nc.vector.scalar_tensor_tensor(
                out=pd2[:], in0=hB[:], scalar=1.0, in1=dec_b[:, HH:],
                op0=mybir.AluOpType.mult, op1=mybir.AluOpType.mult,
                accum_out=sc_b[:],
            )
            nc.scalar.activation(
                out=w_all[:, LAST : LAST + 1], in_=sc_b[:],
                func=mybir.ActivationFunctionType.Exp, bias=sc_a[:],
            )
            emit_mms(LAST)

            # normalizer partial: per-partition sum of the 16 weights
            wsum_sb = singles.tile([P, 1], f32)
            nc.vector.tensor_reduce(
                out=wsum_sb[:], in_=w_all[:],
                axis=mybir.AxisListType.X, op=mybir.AluOpType.add,
            )
            nc.scalar.dma_start(out=wsum_d[:], in_=wsum_sb[:])

            out_sb = singles.tile([1, H], f32)
            nc.vector.tensor_copy(out_sb[:, 0 * HB : 1 * HB], ctx_psum[0][:])
            nc.scalar.copy(out_sb[:, 1 * HB : 2 * HB], ctx_psum[1][:])
            nc.vector.tensor_copy(out_sb[:, 2 * HB : 3 * HB], ctx_psum[2][:])
            nc.scalar.copy(out_sb[:, 3 * HB : 4 * HB], ctx_psum[3][:])
            nc.sync.dma_start(out=out_d[:], in_=out_sb[:])

    nc.compile()
    return nc


def _make_runner(nc):
    """Cached equivalent of bass2jax.run_bass_via_pjrt's multi-core path:
    build the sharded jitted executable once so warm calls skip re-tracing."""
    import jax
    import numpy as np
    from jax.experimental.shard_map import shard_map
    from jax.sharding import Mesh, PartitionSpec

    from concourse import bass2jax, mybir

    bass2jax.install_neuronx_cc_hook()
    assert nc.dbg_addr is None
    partition_name = nc.partition_id_tensor.name if nc.partition_id_tensor else None

    in_names, out_names, out_avals = [], [], []
    for alloc in nc.m.functions[0].allocations:
        if not isinstance(alloc, mybir.MemoryLocationSet):
            continue
        name = alloc.memorylocations[0].name
        if alloc.kind == "ExternalInput":
            if name != partition_name:
                in_names.append(name)
        elif alloc.kind == "ExternalOutput":
            out_names.append(name)
            out_avals.append(
                jax.core.ShapedArray(
                    tuple(alloc.tensor_shape), mybir.dt.np(alloc.dtype)
                )
            )
    n_params = len(in_names)
    all_in = list(in_names) + list(out_names)
    if partition_name is not None:
        all_in.append(partition_name)
    donate = tuple(range(n_params, n_params + len(out_names)))

    def _body(*args):
        operands = list(args)
        if partition_name is not None:
            operands.append(bass2jax.partition_id_tensor())
        return tuple(
            bass2jax._bass_exec_p.bind(
                *operands,
                out_avals=tuple(out_avals),
                in_names=tuple(all_in),
                out_names=tuple(out_names),
                lowering_input_output_aliases=(),
                sim_require_finite=True,
                sim_require_nnan=True,
                nc=nc,
            )
        )

    devices = jax.devices()[:N_CORES]
    mesh = Mesh(np.asarray(devices), ("core",))
    nio = n_params + len(out_names)
    sharded = jax.jit(
        shard_map(
            _body,
            mesh=mesh,
            in_specs=(PartitionSpec("core"),) * nio,
            out_specs=(PartitionSpec("core"),) * len(out_names),
            check_rep=False,
        ),
        donate_argnums=donate,
        keep_unused=True,
    )

    def run(in_maps):
        concat_in = [
            np.concatenate([m[name] for m in in_maps], axis=0) for name in in_names
        ]
        concat_zeros = [
            np.zeros((N_CORES * a.shape[0], *a.shape[1:]), a.dtype)
            for a in out_avals
        ]
        out_arrs = sharded(*concat_in, *concat_zeros)
        return [
            {
                name: np.asarray(out_arrs[i]).reshape(
                    N_CORES, *out_avals[i].shape
                )[c]
                for i, name in enumerate(out_names)
            }
            for c in range(N_CORES)
        ]

    return run


def _run(encoder_hiddens, decoder_hidden, trace=False, **kw):
    from concourse.bass_utils import run_bass_kernel_spmd

    key = "nc_v3"
    if key not in _CACHE:
        _CACHE[key] = _build()
    nc = _CACHE[key]

    enc = np.ascontiguousarray(encoder_hiddens, dtype=np.float16)
    dec = np.ascontiguousarray(decoder_hidden, dtype=np.float16)
    in_maps = [
        {"enc": enc[c * S_LOCAL : (c + 1) * S_LOCAL], "dec": dec}
        for c in range(N_CORES)
    ]
    if trace:
        res = run_bass_kernel_spmd(
            nc, in_maps, core_ids=list(range(N_CORES)), trace=True, **kw
        )
        results = res.results
    else:
        rkey = "runner_v3"
        if rkey not in _CACHE:
            _CACHE[rkey] = _make_runner(nc)
        results = _CACHE[rkey](in_maps)
        res = None

    ctx = np.zeros((1, H), np.float64)
    z = 0.0
    for r in results:
        ctx += r["out"].astype(np.float64)
        z += float(r["wsum"].sum(dtype=np.float64))
    return (ctx / z).astype(np.float32), res


def kernel(encoder_hiddens, decoder_hidden):
    out, _ = _run(encoder_hiddens, decoder_hidden)
    return out
